# revision 11
# baseline (speedup 1.0000x reference)
"""MoE FFN kernel v9 for Trainium2 (Bass/Tile), data-parallel over tokens.

v4 + three changes that cut per-execute operand bytes and device DMA:
 - w2 ships as fp8 e3m4 wire format (scale W2S, clip +-15.4), packed as
   raw bytes inside the bf16 blob; on device the SBUF tile is bitcast to
   float8e3 and fed directly as the matmul stationary operand (PE runs
   fp8e3 at bf16 speed; descale folded into the gelu activation scale).
 - Output is bf16 (unpack casts back to f32).
 - w2 and lin2 weights are streamed from HBM exactly once (f2/ht-outer,
   th-inner loops; tT/hT hold the full 1024 tokens); x_lo is streamed
   per k-tile during the router pass so SBUF fits.

Blob column layout (per-partition bf16-sized columns):
  xb [KO*T] @0, xr [KO*T], wb/wr [KO*E] router_w hi/lo, w1 [E*KO*R],
  w2 [FT2*W2CH] fp8e3 bytes (16 chunks), l2 [HT*FT*P], l2bh/l2bl [HT],
  sel [E*P] (partitions 0..7), rbh/rbl. NCOL = 99584 (25.5 MB/core).
"""

import numpy as np
import ml_dtypes

P = 128
B, S, H, R, F, E = 4, 2048, 1024, 256, 4096, 8
NCORES = 8
TT = B * S
T = TT // NCORES        # 1024 tokens per core
TH = 512
NH = T // TH
KO = H // P             # 8
RO = R // P             # 2
FT = F // P             # 32
HT = H // P             # 8
FT2 = FT // 2           # 16 w2 DMA chunks

W2CH = E * RO * P          # bf16 cols per w2 f2-chunk (2x fp8 elems)
W2S = 136.0                # fp8e3 wire scale for w2
O_XB = 0
O_XR = O_XB + KO * T
O_WB = O_XR + KO * T
O_WR = O_WB + KO * E
O_W1 = O_WR + KO * E
O_W2 = O_W1 + E * KO * R
O_L2 = O_W2 + FT2 * W2CH
O_L2BH = O_L2 + HT * FT * P
O_L2BL = O_L2BH + HT
O_SEL = ((O_L2BL + HT + 63) // 64) * 64
O_RBH = O_SEL + E * P
O_RBL = O_RBH + 1
NCOL = ((O_RBL + 1 + 63) // 64) * 64

_CACHE: dict = {}


def _build_nc(act_name: str = "Gelu"):
    import concourse.mybir as mybir
    import concourse.tile as tile
    from concourse import bacc

    dt = mybir.dt
    f32, bf16 = dt.float32, dt.bfloat16
    Alu = mybir.AluOpType
    Act = mybir.ActivationFunctionType

    nc = bacc.Bacc(
        "TRN2", target_bir_lowering=False, debug=False, enable_asserts=False
    )

    blob = nc.dram_tensor("blob", [P, NCOL], bf16, kind="ExternalInput").ap()
    out = nc.dram_tensor("o", [P, HT, T], bf16, kind="ExternalOutput").ap()

    NEG = -1.0e30

    with tile.TileContext(nc) as tc:
        from contextlib import ExitStack

        with ExitStack() as ctx:
            res = ctx.enter_context(tc.tile_pool(name="res", bufs=1))
            smp = ctx.enter_context(tc.tile_pool(name="smp", bufs=1))
            mwbp = ctx.enter_context(tc.tile_pool(name="mwbp", bufs=2))
            w2p = ctx.enter_context(tc.tile_pool(name="w2p", bufs=2))
            l2p = ctx.enter_context(tc.tile_pool(name="l2p", bufs=2))
            outp = ctx.enter_context(tc.tile_pool(name="outp", bufs=2))
            xrs = ctx.enter_context(tc.tile_pool(name="xrs", bufs=2))
            psp = ctx.enter_context(tc.tile_pool(name="psp", bufs=4, space="PSUM"))
            pslp = ctx.enter_context(tc.tile_pool(name="pslp", bufs=1, space="PSUM"))
            psbp = ctx.enter_context(tc.tile_pool(name="psbp", bufs=2, space="PSUM"))

            def seg(a, n):
                return blob[:, a:a + n]

            # ---- resident loads ----
            xb_sb = res.tile([P, KO, T], bf16)
            nc.sync.dma_start(
                xb_sb, seg(O_XB, KO * T).rearrange("p (k t) -> p k t", k=KO)
            )
            wb_sb = res.tile([P, KO, E], bf16)
            nc.sync.dma_start(
                wb_sb, seg(O_WB, KO * E).rearrange("p (k e) -> p k e", k=KO)
            )
            wr_sb = res.tile([P, KO, E], bf16)
            nc.sync.dma_start(
                wr_sb, seg(O_WR, KO * E).rearrange("p (k e) -> p k e", k=KO)
            )
            w1_sb = res.tile([P, E, KO, R], bf16)
            nc.sync.dma_start(
                w1_sb,
                seg(O_W1, E * KO * R).rearrange("p (e k r) -> p e k r",
                                                e=E, k=KO),
            )
            l2bhl_sb = res.tile([P, 2, HT], bf16)
            nc.sync.dma_start(
                l2bhl_sb,
                seg(O_L2BH, 2 * HT).rearrange("p (x h) -> p x h", x=2),
            )
            l2b_sb = res.tile([P, HT], f32)
            nc.vector.tensor_tensor(
                l2b_sb, l2bhl_sb[:, 0, :], l2bhl_sb[:, 1, :], Alu.add
            )
            sel_sb = res.tile([E, E, P], bf16)
            nc.sync.dma_start(
                sel_sb,
                blob[0:E, O_SEL:O_SEL + E * P].rearrange(
                    "p (e c) -> p e c", e=E
                ),
            )
            rbhl_sb = res.tile([E, 2], bf16)
            nc.sync.dma_start(rbhl_sb, blob[0:E, O_RBH:O_RBH + 2])
            rb_sb = res.tile([E, 1], f32)
            nc.vector.tensor_tensor(
                rb_sb, rbhl_sb[:, 0:1], rbhl_sb[:, 1:2], Alu.add
            )

            tT_sb = res.tile([P, E * RO, T], bf16)
            hT_sb = res.tile([P, FT, T], bf16)

            # ---- router: logits [E, T] in ~fp32 via hi/lo bf16 ----
            # x_lo streamed per k-tile (keeps SBUF small)
            psl = pslp.tile([E, T], f32)
            groups = [(False, wb_sb), (True, wb_sb), (False, wr_sb)]
            for gi, (is_xr, ww) in enumerate(groups):
                for k in range(KO):
                    if is_xr:
                        xrt = xrs.tile([P, T], bf16)
                        nc.sync.dma_start(xrt, seg(O_XR + k * T, T))
                        xk = xrt[:, :]
                    else:
                        xk = xb_sb[:, k, :]
                    for t in range(T // 512):
                        nc.tensor.matmul(
                            psl[:, t * 512:(t + 1) * 512],
                            ww[:, k, :],
                            xk[:, t * 512:(t + 1) * 512],
                            start=(gi == 0 and k == 0),
                            stop=(gi == 2 and k == KO - 1),
                        )

            # ---- softmax + top-2 via DVE shuffle butterflies ----
            def xor_mask(k):
                return [i ^ k for i in range(32)]

            def butterfly(src, op, rtag):
                cur = src
                for k, tag in ((4, "nx4"), (2, "nx2"), (1, rtag)):
                    sh = smp.tile([32, TH], f32, tag=f"sh{k}")
                    nc.vector.stream_shuffle(sh, cur, xor_mask(k))
                    nxt = smp.tile([32, TH], f32, tag=tag)
                    nc.vector.tensor_tensor(nxt, cur, sh, op)
                    cur = nxt
                return cur

            def softmax_top2(tsl):
                lg = smp.tile([32, TH], f32, tag="lg")
                nc.vector.memset(lg, NEG)
                nc.vector.tensor_scalar(
                    lg[0:E, :], psl[:, tsl], rb_sb[:, 0:1], None, op0=Alu.add
                )
                m1 = butterfly(lg, Alu.max, "r1")
                scr = smp.tile([32, TH], f32, tag="scr")
                nc.vector.tensor_tensor(scr, lg, m1, Alu.is_equal)
                nc.vector.tensor_scalar(scr, scr, NEG, None, op0=Alu.mult)
                nc.vector.tensor_tensor(scr, lg, scr, Alu.add)
                sub = smp.tile([32, TH], f32, tag="sub")
                nc.vector.tensor_tensor(sub, lg, m1, Alu.subtract)
                m2 = butterfly(scr, Alu.max, "r2")
                nc.vector.tensor_tensor(scr, lg, m2, Alu.is_ge)  # mask
                ex = smp.tile([32, TH], f32, tag="ex")
                nc.scalar.activation(ex, sub, Act.Exp)
                ssum = butterfly(ex, Alu.add, "r3")
                rcp = smp.tile([32, TH], f32, tag="r1")
                nc.vector.reciprocal(rcp, ssum)
                nc.vector.tensor_tensor(ex, ex, scr, Alu.mult)
                mw = smp.tile([32, TH], f32, tag="r2")
                nc.vector.tensor_tensor(mw, ex, rcp, Alu.mult)
                mwh = smp.tile([32, TH], bf16, tag="mwh")
                nc.vector.tensor_copy(mwh[0:E, :], mw[0:E, :])
                mwhf = smp.tile([32, TH], f32, tag="r3")
                nc.vector.tensor_copy(mwhf[0:E, :], mwh[0:E, :])
                nc.vector.tensor_tensor(
                    mwhf[0:E, :], mw[0:E, :], mwhf[0:E, :], Alu.subtract
                )
                mwl = smp.tile([32, TH], bf16, tag="mwl")
                nc.vector.tensor_copy(mwl[0:E, :], mwhf[0:E, :])
                return mwh, mwl

            for th in range(NH):
                tsl = slice(th * TH, (th + 1) * TH)
                mwh, mwl = softmax_top2(tsl)

                # ---- experts: tT[e*RO+r] = (w1_e.T @ x) * mw_e ----
                for e in range(E):
                    psb = psbp.tile([P, TH], f32, tag="psb")
                    nc.tensor.matmul(
                        psb, sel_sb[:, e, :], mwh[0:E, :],
                        start=True, stop=False,
                    )
                    nc.tensor.matmul(
                        psb, sel_sb[:, e, :], mwl[0:E, :],
                        start=False, stop=True,
                    )
                    mwb = mwbp.tile([P, TH], f32)
                    nc.vector.tensor_copy(mwb, psb)
                    for r in range(RO):
                        pst = psp.tile([P, TH], f32, tag="ps")
                        for k in range(KO):
                            nc.tensor.matmul(
                                pst,
                                w1_sb[:, e, k, r * P:(r + 1) * P],
                                xb_sb[:, k, tsl],
                                start=(k == 0),
                                stop=(k == KO - 1),
                            )
                        nc.vector.tensor_tensor(
                            tT_sb[:, e * RO + r, tsl], pst, mwb, Alu.mult
                        )

            # ---- mixedT [F, T] + gelu -> hT bf16 (w2 streamed once) ----
            for f2 in range(FT2):
                w2t = w2p.tile([P, W2CH], bf16)
                nc.sync.dma_start(
                    w2t, seg(O_W2 + f2 * W2CH, W2CH)
                )
                w2v = w2t[:, :].bitcast(dt.float8e3).rearrange(
                    "p (h e r c) -> p h e r c", h=2, e=E, r=RO
                )
                for half in range(2):
                    for th in range(NH):
                        tsl = slice(th * TH, (th + 1) * TH)
                        psf = psp.tile([P, TH], f32, tag="ps")
                        for e in range(E):
                            for r in range(RO):
                                nc.tensor.matmul(
                                    psf,
                                    w2v[:, half, e, r, :],
                                    tT_sb[:, e * RO + r, tsl],
                                    start=(e == 0 and r == 0),
                                    stop=(e == E - 1 and r == RO - 1),
                                )
                        nc.scalar.activation(
                            hT_sb[:, f2 * 2 + half, tsl], psf,
                            getattr(Act, act_name), scale=1.0 / W2S
                        )

            # ---- outT [H, T] = lin2.T @ h + b (l2 streamed once) ----
            for ht in range(HT):
                l2t = l2p.tile([P, FT, P], bf16)
                nc.sync.dma_start(
                    l2t,
                    seg(O_L2 + ht * FT * P, FT * P).rearrange(
                        "p (f c) -> p f c", f=FT
                    ),
                )
                for th in range(NH):
                    tsl = slice(th * TH, (th + 1) * TH)
                    pso = psp.tile([P, TH], f32, tag="ps")
                    for ko in range(FT):
                        nc.tensor.matmul(
                            pso,
                            l2t[:, ko, :],
                            hT_sb[:, ko, tsl],
                            start=(ko == 0),
                            stop=(ko == FT - 1),
                        )
                    ot = outp.tile([P, TH], bf16)
                    nc.vector.tensor_scalar(
                        ot, pso, l2b_sb[:, ht:ht + 1], None, op0=Alu.add
                    )
                    nc.sync.dma_start(out[:, ht, tsl], ot)

    nc.compile()
    return nc


def get_nc(act_name: str = "Gelu"):
    key = f"nc_{act_name}"
    if key not in _CACHE:
        _CACHE[key] = _build_nc(act_name)
    return _CACHE[key]


def _hi_lo(a):
    bf = ml_dtypes.bfloat16
    hi = a.astype(bf)
    lo = (a - hi.astype(np.float32)).astype(bf)
    return hi, lo


def pack_inputs(inputs):
    """Full-problem numpy inputs -> list of 8 per-core {"blob": ...}."""
    bf = ml_dtypes.bfloat16
    x = np.asarray(inputs["x"], np.float32).reshape(TT, H)
    router_w = np.asarray(inputs["router_w"], np.float32)
    router_b = np.asarray(inputs["router_b"], np.float32)
    w1 = np.asarray(inputs["w1"], np.float32)
    w2 = np.asarray(inputs["w2"], np.float32)
    lin2_w = np.asarray(inputs["lin2_w"], np.float32)
    lin2_b = np.asarray(inputs["lin2_b"], np.float32)

    shared = np.zeros((P, NCOL - O_WB), bf)   # columns from O_WB on
    off = -O_WB

    wbh, wbr = _hi_lo(router_w)
    shared[:, O_WB + off:O_WB + off + KO * E] = (
        wbh.reshape(KO, P, E).transpose(1, 0, 2).reshape(P, KO * E)
    )
    shared[:, O_WR + off:O_WR + off + KO * E] = (
        wbr.reshape(KO, P, E).transpose(1, 0, 2).reshape(P, KO * E)
    )
    shared[:, O_W1 + off:O_W1 + off + E * KO * R] = (
        w1.astype(bf).reshape(E, KO, P, R).transpose(2, 0, 1, 3)
        .reshape(P, E * KO * R)
    )
    # [E, R, F] -> fp8e3 bytes [P, FT2, 2, E, RO, C] viewed as bf16 cols
    w2q = np.clip(w2 * W2S, -15.4, 15.4).astype(ml_dtypes.float8_e3m4)
    w2b = (
        w2q.reshape(E, RO, P, FT2, 2, P)
        .transpose(2, 3, 4, 0, 1, 5).copy().view(np.uint8)
        .reshape(P, FT2 * 2 * E * RO * P).view(bf)
    )
    shared[:, O_W2 + off:O_W2 + off + FT2 * W2CH] = w2b
    # [F, H] -> [P, HT, FT, C]
    shared[:, O_L2 + off:O_L2 + off + HT * FT * P] = (
        lin2_w.astype(bf).reshape(FT, P, HT, P).transpose(1, 2, 0, 3)
        .reshape(P, HT * FT * P)
    )
    l2bh, l2bl = _hi_lo(lin2_b)
    shared[:, O_L2BH + off:O_L2BH + off + HT] = l2bh.reshape(HT, P).T
    shared[:, O_L2BL + off:O_L2BL + off + HT] = l2bl.reshape(HT, P).T
    for e in range(E):
        shared[e, O_SEL + off + e * P:O_SEL + off + (e + 1) * P] = 1.0
    rbh, rbl = _hi_lo(router_b)
    shared[0:E, O_RBH + off] = rbh
    shared[0:E, O_RBL + off] = rbl

    in_maps = []
    for c in range(NCORES):
        xt = x[c * T:(c + 1) * T].T  # [H, T]
        xth, xtl = _hi_lo(xt)
        blob = np.empty((P, NCOL), bf)
        blob[:, O_XB:O_XB + KO * T] = (
            xth.reshape(KO, P, T).transpose(1, 0, 2).reshape(P, KO * T)
        )
        blob[:, O_XR:O_XR + KO * T] = (
            xtl.reshape(KO, P, T).transpose(1, 0, 2).reshape(P, KO * T)
        )
        blob[:, O_WB:] = shared
        in_maps.append({"blob": blob})
    return in_maps


def unpack_outputs(outs):
    """list of 8 per-core [P, HT, T] f32 -> [B, S, H] f32."""
    parts = []
    for o in outs:
        oc = np.asarray(o).astype(np.float32).transpose(2, 1, 0).reshape(T, H)
        parts.append(oc)
    return np.concatenate(parts, axis=0).reshape(B, S, H)


def kernel(**inputs) -> np.ndarray:
    from concourse import bass_utils

    nc = get_nc()
    in_maps = pack_inputs(inputs)
    res = bass_utils.run_bass_kernel_spmd(
        nc, in_maps, core_ids=list(range(NCORES))
    )
    return unpack_outputs([r["o"] for r in res.results])

